# revision 23
# baseline (speedup 1.0000x reference)
"""CodeWiseAttention kernel for Trainium2 (8 NeuronCores, label-dim sharded).

m[b,n,:] = softmax(label_feature[n] @ x[b].T) @ x[b]

Sharding: label rows N=8922 split across 8 cores (1116/core; core 7 pads 6
rows); x replicated. All transposes/padding done on host; on-chip per core,
one global stream of 160 l-chunks (8 batches x 20 chunks of 128 rows):
  mm1 (fp16):  S[l,n] = xT[e,l].T @ labT[e,n]   3 matmuls of n-width 372
  exp (ScalarE): e_sb = bf16(exp(S - 30))       one call per l-chunk (N=1116)
  mm2 (bf16):  m_ps[n,e'] += e_sb[l,n-chunk].T @ xa[l,e']  9 matmuls, lagging
      the exp stream by 2 chunks; xa has a ones column so col 100 of U = Z.
  out: m = U/Z on VectorE straight from [n,e] layout, 2 DMA triggers/batch.

ScalarE's exp stream (1 elem/cycle/lane, cost = free-dim columns) is the
roofline: ~152us of pure streaming + per-call overhead. Everything else is
shaped to keep ScalarE gapless: mm1 of chunk g is emitted BEFORE mm2 of
chunk g-2 so the PE (in-order queue) produces scores the moment the s-buffer
frees; weight loads hide under matmuls via the PE's reorder window; input
DMAs trigger from the idle GpSimd queue and outputs from Sync (each trigger
costs ~0.6us of sequencer time); VectorE does the U/Z division.

fp16 for mm1 keeps score error ~4e-3 abs (exp amplifies score errors); bf16
for mm2 because exp values reach e^31 (fp16 would overflow). Measured rel
err vs f64 reference: ~4e-3 (threshold 2e-2).

The 9 m-accumulator slots (102 f32 cols) pack 5-per-bank into 2 PSUM banks.
Only the first slot of each bank uses start=True (clearing the whole bank's
has_written bits); the other slots' first matmuls then overwrite-where-unset
and all later chunks accumulate — 9 interleaved accumulation groups share 2
banks. The per-batch U snapshot to SBUF doubles as the WAR anchor that keeps
the next batch's matmuls out of m_ps while VectorE reads it (PE-write +
DVE-read of one PSUM bank is fatal; reading uninitialized PSUM is fatal too,
hence the one-time memset). PSUM: 2 x 3 banks scores + 2 banks U = 8.
"""
import numpy as np
from contextlib import ExitStack

import concourse.tile as tile
from concourse import bacc, mybir
from concourse.bass_utils import run_bass_kernel_spmd

F32 = mybir.dt.float32
F16 = mybir.dt.float16
BF16 = mybir.dt.bfloat16

B, L, E = 8, 2500, 100
LP = 2560          # L padded to 20 chunks of 128 (zero rows contribute nothing)
NCHUNK = LP // 128
NG = B * NCHUNK
N_TOTAL = 8922
NCORES = 8
NS = 1116          # label rows per core (core 7: 1110 real, 6 pad)
NCH = 372          # mm1 moving width (3 x 372 = 1116)
EA = E + 2         # x cols + ones col (100) + pad col (101)
PSB = 512          # psum bank stride in f32 elements
EXP_BIAS = -30.0
SLOT_P = [128] * 8 + [92]   # n-rows per m-accumulator slot (9 x 128 > 1116)

TRACE = False
LAST_RESULT = None

_NC = []


def _build():
    nc = bacc.Bacc("TRN2", target_bir_lowering=False, debug=False)
    xt_d = nc.dram_tensor("xt", [B, E, LP], F16, kind="ExternalInput").ap()
    xa_d = nc.dram_tensor("xa", [B, LP, EA], BF16, kind="ExternalInput").ap()
    lab_d = nc.dram_tensor("lab", [E, NS], F16, kind="ExternalInput").ap()
    m_d = nc.dram_tensor("m", [B, NS, E], F32, kind="ExternalOutput").ap()

    with tile.TileContext(nc) as tc, ExitStack() as ctx:
        consts = ctx.enter_context(tc.tile_pool(name="consts", bufs=1))
        xt_pool = ctx.enter_context(tc.tile_pool(name="xtp", bufs=2))
        xa_pool = ctx.enter_context(tc.tile_pool(name="xap", bufs=2))
        e_pool = ctx.enter_context(tc.tile_pool(name="ep", bufs=4))
        u_pool = ctx.enter_context(tc.tile_pool(name="up", bufs=2))
        o_pool = ctx.enter_context(tc.tile_pool(name="op", bufs=2))
        r_pool = ctx.enter_context(tc.tile_pool(name="rp", bufs=2))
        s_psum = ctx.enter_context(tc.tile_pool(name="sps", bufs=2, space="PSUM"))
        m_psum = ctx.enter_context(tc.tile_pool(name="mps", bufs=1, space="PSUM"))

        labT_sb = consts.tile([E, NS], F16)
        warm_sb = consts.tile([128, 128], F16)
        nc.vector.memset(warm_sb[:], 0.0)
        bias_sb = consts.tile([128, 1], F32)
        nc.vector.memset(bias_sb[:], EXP_BIAS)

        m_ps = m_psum.tile([128, 2, PSB], F32, name="m_ps")
        # one-time init so whole-tile reads below never touch uninitialized
        # PSUM; 1.0 (not 0) so the dummy-slot 1/Z stays finite
        nc.vector.memset(m_ps[:], 1.0)

        xt_tiles, xa_tiles, e_tiles = {}, {}, {}

        def dma_in(b):
            xt_tiles[b] = xt_pool.tile([E, LP], F16, tag="xt", name=f"xt{b}")
            xa_tiles[b] = xa_pool.tile(
                [128, NCHUNK, EA], BF16, tag="xa", name=f"xa{b}")
            nc.gpsimd.dma_start(out=xt_tiles[b][:], in_=xt_d[b])
            nc.gpsimd.dma_start(
                out=xa_tiles[b][:],
                in_=xa_d[b].rearrange("(c p) e -> p c e", p=128))

        def mm2_chunk(b, c, e_sb):
            xa_sb = xa_tiles[b]
            e_flat = e_sb[:].rearrange("p a b -> p (a b)")
            for jn in range(9):
                q, r2 = divmod(jn, 5)
                np_ = SLOT_P[jn]
                nc.tensor.matmul(
                    m_ps[0:np_, q, r2 * EA:(r2 + 1) * EA],
                    e_flat[:, jn * 128:jn * 128 + np_],
                    xa_sb[:, c, :],
                    start=(c == 0 and r2 == 0), stop=(c == NCHUNK - 1),
                    skip_group_check=True,
                )

        def out_path(b, src):
            # strided reciprocals over the Z columns (slot (1,4) is a
            # dummy), broadcast multiplies for U/Z, then DMA
            u4 = src[:, :, 0:EA * 5].rearrange(
                "p q (r c) -> p q r c", r=5)[:, :, :, 0:E]
            o_flat = o_pool.tile([128, 10 * E], F32, tag="o")
            o4 = o_flat[:].rearrange("p (q r c) -> p q r c", q=2, r=5)
            o3 = o_flat[:].rearrange("p (s c) -> p s c", s=10)
            if b + 1 < B:
                rz = r_pool.tile([128, 2, 5], F32, tag="r")
                nc.vector.reciprocal(rz[:], src[:, :, E:EA * 5:EA])
                nc.vector.tensor_tensor(
                    o4, u4, rz[:].to_broadcast([128, 2, 5, E]),
                    mybir.AluOpType.mult)
                nc.sync.dma_start(
                    out=m_d[b, 0:1024, :].rearrange(
                        "(s p) e -> p s e", p=128),
                    in_=o3[:, 0:8, :])
                nc.sync.dma_start(
                    out=m_d[b, 1024:NS, :], in_=o3[0:NS - 1024, 8, :])
            else:
                # last batch is the exposed tail: process per bank so bank
                # 0's reciprocal+multiply+DMA-trigger run while the PE is
                # still filling bank 1's slots (different banks, so the
                # DVE-read/PE-write hazard doesn't apply), with the trailing
                # transfers split across the Sync and GpSimd queues
                rz0 = r_pool.tile([128, 5], F32, tag="r0")
                nc.vector.reciprocal(rz0[:], src[:, 0, E:EA * 5:EA])
                nc.vector.tensor_tensor(
                    o4[:, 0], u4[:, 0], rz0[:].to_broadcast([128, 5, E]),
                    mybir.AluOpType.mult)
                nc.sync.dma_start(
                    out=m_d[b, 0:640, :].rearrange("(s p) e -> p s e", p=128),
                    in_=o3[:, 0:5, :])
                rz1 = r_pool.tile([128, 5], F32, tag="r1")
                nc.vector.reciprocal(rz1[:], src[:, 1, E:EA * 5:EA])
                nc.vector.tensor_tensor(
                    o4[:, 1], u4[:, 1], rz1[:].to_broadcast([128, 5, E]),
                    mybir.AluOpType.mult)
                nc.gpsimd.dma_start(
                    out=m_d[b, 640:1024, :].rearrange(
                        "(s p) e -> p s e", p=128),
                    in_=o3[:, 5:8, :])
                nc.sync.dma_start(
                    out=m_d[b, 1024:NS, :], in_=o3[0:NS - 1024, 8, :])

        def finish_batch(b):
            if b + 1 < B:
                # U snapshot: the WAR anchor keeping batch b+1's matmuls out
                # of m_ps until VectorE has read batch b's result
                u_sb = u_pool.tile([128, 2, PSB], F32, tag="u")
                nc.vector.tensor_copy(u_sb[:], m_ps[:])
                out_path(b, u_sb)
            else:
                out_path(b, m_ps)  # last batch: no later matmuls
            del xa_tiles[b]

        # prologue: chunk 0's deps (xt piece 1, labT) trigger first, labT
        # halves split across the Sync and GpSimd queues so the two ring
        # writes and transfers run in parallel
        xt_tiles[0] = xt_pool.tile([E, LP], F16, tag="xt", name="xt0")
        xa_tiles[0] = xa_pool.tile(
            [128, NCHUNK, EA], BF16, tag="xa", name="xa0")
        xa0_r = xa_d[0].rearrange("(c p) e -> p c e", p=128)
        nc.sync.dma_start(out=xt_tiles[0][:, 0:128], in_=xt_d[0][:, 0:128])
        nc.gpsimd.dma_start(out=labT_sb[:, 0:558], in_=lab_d[:, 0:558])
        nc.sync.dma_start(out=labT_sb[:, 558:NS], in_=lab_d[:, 558:NS])
        nc.gpsimd.dma_start(out=xa_tiles[0][:, 0:2, :], in_=xa0_r[:, 0:2, :])
        nc.gpsimd.dma_start(
            out=xt_tiles[0][:, 128:640], in_=xt_d[0][:, 128:640])
        nc.gpsimd.dma_start(out=xa_tiles[0][:, 2:6, :], in_=xa0_r[:, 2:6, :])
        nc.gpsimd.dma_start(
            out=xt_tiles[0][:, 640:1536], in_=xt_d[0][:, 640:1536])
        nc.gpsimd.dma_start(out=xa_tiles[0][:, 6:13, :], in_=xa0_r[:, 6:13, :])
        nc.gpsimd.dma_start(
            out=xt_tiles[0][:, 1536:LP], in_=xt_d[0][:, 1536:LP])
        nc.gpsimd.dma_start(
            out=xa_tiles[0][:, 13:NCHUNK, :], in_=xa0_r[:, 13:NCHUNK, :])

        # PE warm-up during the prologue DMA wait: ~2.4us of dummy matmuls
        # keeps the HAM activity window busy so the clock gate flips to
        # 2.4 GHz before (not during) batch 0. The garbage lands inside
        # m-accumulator slot regions that batch 0's start=True protocol
        # fully overwrites; the memset-initialized gap columns are untouched.
        for _ in range(22):
            nc.tensor.matmul(
                m_ps[:, 1, 128:256], warm_sb[:], warm_sb[:],
                start=True, stop=True, skip_group_check=True)
        for g in range(NG + 2):
            b, c = divmod(g, NCHUNK)
            if g < NG:
                if c == 10 and b + 1 < B:
                    dma_in(b + 1)
                xt_sb = xt_tiles[b]
                s_ps = s_psum.tile([128, 3, PSB], F32, tag="s")
                for j3 in range(3):
                    nc.tensor.matmul(
                        s_ps[:, j3, 0:NCH],
                        xt_sb[:, c * 128:(c + 1) * 128],
                        labT_sb[:, j3 * NCH:(j3 + 1) * NCH],
                    )
                e_tiles[g] = e_pool.tile(
                    [128, 3, NCH], BF16, tag="e", name=f"e{g}")
                nc.scalar.activation(
                    e_tiles[g][:], s_ps[:, 0:3, 0:NCH],
                    mybir.ActivationFunctionType.Exp,
                    bias=bias_sb[:], scale=1.0,
                )
                if c == NCHUNK - 1:
                    del xt_tiles[b]
            if g >= 2:
                b2, c2 = divmod(g - 2, NCHUNK)
                mm2_chunk(b2, c2, e_tiles.pop(g - 2))
                if c2 == NCHUNK - 1:
                    finish_batch(b2)
    nc.compile()
    return nc


def _get_nc():
    if not _NC:
        _NC.append(_build())
    return _NC[0]


def kernel(x, label_feature):
    global LAST_RESULT
    np_f16 = mybir.dt.np(F16)
    np_bf16 = mybir.dt.np(BF16)
    x = np.ascontiguousarray(np.asarray(x, dtype=np.float32))
    lf = np.ascontiguousarray(np.asarray(label_feature, dtype=np.float32))
    assert x.shape == (B, L, E) and lf.shape == (N_TOTAL, E)

    xt = np.zeros((B, E, LP), np_f16)
    xt[:, :, :L] = x.transpose(0, 2, 1).astype(np_f16)
    xa = np.zeros((B, LP, EA), np_bf16)
    xa[:, :L, :E] = x.astype(np_bf16)
    xa[:, :L, E] = 1.0

    in_maps = []
    for r in range(NCORES):
        lo = r * NS
        hi = min(lo + NS, N_TOTAL)
        shard = np.zeros((E, NS), np_f16)
        shard[:, : hi - lo] = lf[lo:hi].T.astype(np_f16)
        in_maps.append({"xt": xt, "xa": xa, "lab": shard})

    nc = _get_nc()
    res = run_bass_kernel_spmd(
        nc, in_maps, core_ids=list(range(NCORES)), trace=TRACE
    )
    LAST_RESULT = res

    out = np.empty((B, N_TOTAL, E), np.float32)
    for r in range(NCORES):
        lo = r * NS
        hi = min(lo + NS, N_TOTAL)
        out[:, lo:hi, :] = res.results[r]["m"][:, : hi - lo, :]
    return out
